# revision 20
# baseline (speedup 1.0000x reference)
"""Trainium2 Bass kernel for a single-layer transformer encoder block.

Strategy: pure data parallelism — the batch dim (8) maps 1:1 onto the 8
NeuronCores; each core runs the full encoder block on its [1024, 768] slice.
No collectives needed.

Per-core dataflow (T=1024 tokens, C=768, H=12 heads, hs=64, F=3072):
  LN1 (token-major) -> transpose to feature-major hT -> q/k/v projections
  (qT/kT feature-major, v token-major with a fused ones-column for the
  softmax denominator) -> per-head S^T = k q^T (two heads packed in the
  128x128 PE array via tile_position row tiling, K=64 each) -> exp on ACT
  (scale 1/sqrt(C) fused) -> oT = [v|1]^T exp (denominator lands in
  PSUM row 64) -> normalize via gpsimd partition_broadcast + DVE multiply
  -> proj (token-major) + residual -> LN2 -> FFN (f-chunked, relu+bias
  fused into the ACT PSUM->SBUF copy) -> + residual -> DMA out.

All matmuls run with float32 data bitcast to float32r (full-rate single-pass
fp32 on the PE for moving dims >= 256).
"""

import sys

for _p in ("/opt/trn_rl_repo", "/root/.axon_site/_ro/trn_rl_repo"):
    if _p not in sys.path:
        sys.path.append(_p)

import numpy as np

import concourse.bass as bass
import concourse.bacc as bacc
import concourse.mybir as mybir
import concourse.tile as tile
from concourse import masks
from concourse import library_config
from concourse.bass_utils import run_bass_kernel_spmd

F32 = mybir.dt.float32
F32R = mybir.dt.float32r
AF = mybir.ActivationFunctionType
ALU = mybir.AluOpType

B = 8
T = 1024
C = 768
H = 12
HS = 64
F = 3072
EPS = 1e-5
SCALE = 1.0 / float(np.sqrt(C))

NT = T // 128  # 8 token tiles
KC = C // 128  # 6 feature chunks
NFC = 4  # FFN f-chunks
FCW = F // NFC  # 768 f columns per chunk


def _bcast_ap(dram_ap, parts=128):
    """DRAM read AP replicated across `parts` partitions (step-0 partition dim)."""
    return bass.AP(
        tensor=dram_ap.tensor,
        offset=dram_ap.offset,
        ap=[[0, parts]] + [list(d) for d in dram_ap.ap],
    )


def _perpart_ap(dram_ap, cols):
    """[N] DRAM vector viewed as [128, cols] with the 128 index innermost:
    element (p, j) = v[j*128 + p]."""
    return bass.AP(
        tensor=dram_ap.tensor,
        offset=dram_ap.offset,
        ap=[[1, 128], [128, cols]],
    )


def split_excess_waits(nc, max_waits=1):
    """This walrus build rejects instructions carrying more than one sem wait
    (seen on the Tile end-drain). Move excess waits onto dedicated NoOps."""
    for f in nc.m.functions:
        for bb in f.blocks:
            insts = list(bb.instructions)
            out = []
            changed = False
            for inst in insts:
                si = inst.sync_info
                if si is not None and si.on_wait and len(si.on_wait) > max_waits:
                    waits = list(si.on_wait)
                    extra, keep = waits[:-max_waits], waits[-max_waits:]
                    for i in range(0, len(extra), max_waits):
                        nop = mybir.InstNoOp(name=f"I-waitsplit-{nc.next_id()}")
                        nop.engine = inst.engine
                        nop.sync_info = mybir.SyncInfo(
                            on_wait=extra[i : i + max_waits], on_update=[]
                        )
                        out.append(nop)
                    inst.sync_info = mybir.SyncInfo(
                        on_wait=keep, on_update=list(si.on_update)
                    )
                    changed = True
                out.append(inst)
            if changed:
                bb.instructions[:] = out


def build_kernel(split_waits=True):
    nc = bacc.Bacc()

    x_d = nc.dram_tensor("x", [T, C], F32, kind="ExternalInput")
    wq_d = nc.dram_tensor("Wq", [H, C, HS], F32R, kind="ExternalInput")
    bq_d = nc.dram_tensor("bq", [H, HS], F32, kind="ExternalInput")
    wk_d = nc.dram_tensor("Wk", [H, C, HS], F32R, kind="ExternalInput")
    bk_d = nc.dram_tensor("bk", [H, HS], F32, kind="ExternalInput")
    wv_d = nc.dram_tensor("Wv", [H, C, HS], F32R, kind="ExternalInput")
    bv_d = nc.dram_tensor("bv", [H, HS], F32, kind="ExternalInput")
    wp_d = nc.dram_tensor("Wp", [C, C], F32R, kind="ExternalInput")
    bp_d = nc.dram_tensor("bp", [C], F32, kind="ExternalInput")
    w1_d = nc.dram_tensor("W1", [C, F], F32R, kind="ExternalInput")
    b1_d = nc.dram_tensor("b1", [F], F32, kind="ExternalInput")
    w2_d = nc.dram_tensor("W2", [F, C], F32R, kind="ExternalInput")
    b2_d = nc.dram_tensor("b2", [C], F32, kind="ExternalInput")
    g1_d = nc.dram_tensor("g1", [C], F32, kind="ExternalInput")
    be1_d = nc.dram_tensor("beta1", [C], F32, kind="ExternalInput")
    g2_d = nc.dram_tensor("g2", [C], F32, kind="ExternalInput")
    be2_d = nc.dram_tensor("beta2", [C], F32, kind="ExternalInput")
    out_d = nc.dram_tensor("out", [T, C], F32, kind="ExternalOutput")

    with tile.TileContext(nc) as tc:
        consts = tc.alloc_tile_pool(name="consts", bufs=1)
        work = tc.alloc_tile_pool(name="work", bufs=2)
        ps = tc.alloc_tile_pool(name="ps", bufs=1, space="PSUM")

        # ---------------- constants ----------------
        ident = consts.tile([128, 128], F32, name="ident")
        masks.make_identity(nc, ident[:])
        nc.gpsimd.load_library(library_config.attn)
        eps_t = consts.tile([128, 1], F32, name="eps_t")
        nc.vector.memset(eps_t[:], EPS)

        g1b = consts.tile([128, C], F32, name="g1b")
        nc.sync.dma_start(out=g1b[:], in_=_bcast_ap(g1_d[:]))
        be1b = consts.tile([128, C], F32, name="be1b")
        nc.sync.dma_start(out=be1b[:], in_=_bcast_ap(be1_d[:]))
        g2b = consts.tile([128, C], F32, name="g2b")
        nc.sync.dma_start(out=g2b[:], in_=_bcast_ap(g2_d[:]))
        be2b = consts.tile([128, C], F32, name="be2b")
        nc.sync.dma_start(out=be2b[:], in_=_bcast_ap(be2_d[:]))
        bpb = consts.tile([128, C], F32, name="bpb")
        nc.sync.dma_start(out=bpb[:], in_=_bcast_ap(bp_d[:]))
        b2b = consts.tile([128, C], F32, name="b2b")
        nc.sync.dma_start(out=b2b[:], in_=_bcast_ap(b2_d[:]))
        bvb = consts.tile([128, C], F32, name="bvb")
        nc.sync.dma_start(
            out=bvb[:], in_=_bcast_ap(bv_d[:, :].rearrange("h d -> (h d)"))
        )

        bq_sb = consts.tile([128, KC], F32, name="bq_sb")
        nc.sync.dma_start(out=bq_sb[:], in_=_perpart_ap(bq_d[:, :], KC))
        bk_sb = consts.tile([128, KC], F32, name="bk_sb")
        nc.sync.dma_start(out=bk_sb[:], in_=_perpart_ap(bk_d[:, :], KC))
        b1_sb = consts.tile([128, F // 128], F32, name="b1_sb")
        nc.sync.dma_start(out=b1_sb[:], in_=_perpart_ap(b1_d[:], F // 128))

        # ---------------- phase 0: load x, LN1 -> h, transpose -> hT ----
        p_h = tc.alloc_tile_pool(name="p_h", bufs=1, side="right")
        p_hT = tc.alloc_tile_pool(name="p_hT", bufs=1, side="right")
        h_t = []
        for i in range(NT):
            xt = work.tile([128, C], F32, name="xt")
            nc.sync.dma_start(out=xt[:], in_=x_d[i * 128 : (i + 1) * 128, :])
            stats = work.tile([128, 3, 6], F32, name="stats")
            for g in range(3):
                nc.vector.bn_stats(
                    out=stats[:, g, :], in_=xt[:, g * 256 : (g + 1) * 256]
                )
            mv = work.tile([128, 2], F32, name="mv")
            nc.vector.bn_aggr(out=mv[:], in_=stats[:])
            rstd = work.tile([128, 1], F32, name="rstd")
            nc.scalar.activation(
                out=rstd[:], in_=mv[:, 1:2], func=AF.Sqrt, bias=eps_t[:]
            )
            nc.vector.reciprocal(out=rstd[:], in_=rstd[:])
            hi = p_h.tile([128, C], F32, name=f"h_{i}")
            nc.vector.tensor_scalar(
                out=hi[:],
                in0=xt[:],
                scalar1=mv[:, 0:1],
                scalar2=rstd[:],
                op0=ALU.subtract,
                op1=ALU.mult,
            )
            nc.vector.tensor_mul(out=hi[:], in0=hi[:], in1=g1b[:])
            nc.vector.tensor_add(out=hi[:], in0=hi[:], in1=be1b[:])
            h_t.append(hi)

        hT = [p_hT.tile([128, T], F32R, name=f"hT_{j}") for j in range(KC)]
        for i in range(NT):
            for j in range(KC):
                pst = ps.tile([128, 128], F32, name="pst", tag="tr", bufs=2)
                nc.tensor.transpose(
                    pst[:], h_t[i][:, j * 128 : (j + 1) * 128], ident[:]
                )
                nc.scalar.activation(
                    out=hT[j][:, i * 128 : (i + 1) * 128], in_=pst[:], func=AF.Copy
                )

        # ---------------- phase 1: q/k/v projections ----------------
        p_wv = tc.alloc_tile_pool(name="p_wv", bufs=1, side="right")
        p_wqk = tc.alloc_tile_pool(name="p_wqk", bufs=1, side="right")
        wq_sb, wk_sb, wv_sb = [], [], []
        for ci in range(KC):
            for nm, d, lst, pool in (
                ("wq", wq_d, wq_sb, p_wqk),
                ("wk", wk_d, wk_sb, p_wqk),
                ("wv", wv_d, wv_sb, p_wv),
            ):
                w = pool.tile([128, H, HS], F32R, name=f"{nm}_{ci}")
                nc.sync.dma_start(
                    out=w[:],
                    in_=d[:, :, :].rearrange("h c d -> c h d")[
                        ci * 128 : (ci + 1) * 128
                    ],
                )
                lst.append(w)

        p_qk = tc.alloc_tile_pool(name="p_qk", bufs=1)
        qT = [p_qk.tile([128, T], F32R, name=f"qT_{j}") for j in range(KC)]
        kT = [p_qk.tile([128, T], F32R, name=f"kT_{j}") for j in range(KC)]
        for w_sb, b_sb, outT in ((wq_sb, bq_sb, qT), (wk_sb, bk_sb, kT)):
            for co in range(KC):
                for tch in range(2):
                    pq = ps.tile([128, 512], F32, name="pq", tag="mm", bufs=2)
                    for ci in range(KC):
                        lhsT = w_sb[ci][:].rearrange("p h d -> p (h d)")[
                            :, co * 128 : (co + 1) * 128
                        ]
                        nc.tensor.matmul(
                            pq[:],
                            lhsT,
                            hT[ci][:, tch * 512 : (tch + 1) * 512],
                            start=(ci == 0),
                            stop=(ci == KC - 1),
                        )
                    nc.scalar.activation(
                        out=outT[co][:, tch * 512 : (tch + 1) * 512],
                        in_=pq[:],
                        func=AF.Identity,
                        bias=b_sb[:, co : co + 1],
                    )
        p_wqk.release()

        # v token-major, heads strided by 65 with a ones column per head
        p_vext = tc.alloc_tile_pool(name="p_vext", bufs=1)
        vext = [p_vext.tile([128, H, 65], F32R, name=f"vext_{i}") for i in range(NT)]
        for i in range(NT):
            for n in range(2):
                pv = ps.tile([128, 512], F32, name="pv", tag="mm", bufs=2)
                for ci in range(KC):
                    nc.tensor.matmul(
                        pv[:, :384],
                        hT[ci][:, i * 128 : (i + 1) * 128],
                        wv_sb[ci][:].rearrange("p h d -> p (h d)")[
                            :, n * 384 : (n + 1) * 384
                        ],
                        start=(ci == 0),
                        stop=(ci == KC - 1),
                    )
                nc.vector.tensor_add(
                    out=vext[i][:, n * 6 : (n + 1) * 6, 0:64],
                    in0=pv[:, :384].rearrange("p (h d) -> p h d", d=64),
                    in1=bvb[:, n * 384 : (n + 1) * 384].rearrange(
                        "p (h d) -> p h d", d=64
                    ),
                )
            nc.vector.memset(vext[i][:, :, 64:65].bitcast(F32), 1.0)
        p_wv.release()
        p_hT.release()

        # ---------------- phase 2: attention ----------------
        p_oT = tc.alloc_tile_pool(name="p_oT", bufs=1, side="right")
        p_wp = tc.alloc_tile_pool(name="p_wp", bufs=1, side="right")
        pexp = tc.alloc_tile_pool(name="pexp", bufs=3)
        pnorm = tc.alloc_tile_pool(name="pnorm", bufs=2)
        wp_sb = []
        for k in range(KC):
            w = p_wp.tile([128, C], F32R, name=f"wp_{k}")
            nc.sync.dma_start(out=w[:], in_=wp_d[k * 128 : (k + 1) * 128, :])
            wp_sb.append(w)

        oT = [p_oT.tile([128, T], F32R, name=f"oT_{j}") for j in range(KC)]
        for jp in range(KC):  # head pair (2*jp, 2*jp+1)
            for tch in range(2):
                o_ps = {}
                o_ps[0] = ps.tile([128, 512], F32, name="o_a", tag="o_a", bufs=1)
                o_ps[1] = ps.tile([128, 512], F32, name="o_b", tag="o_b", bufs=1)
                for st in range(NT):
                    s_a = ps.tile([128, 512], F32, name="s_a", tag="s_a", bufs=1)
                    s_b = ps.tile([128, 512], F32, name="s_b", tag="s_b", bufs=1)
                    nc.tensor.matmul(
                        s_a[:],
                        kT[jp][0:64, st * 128 : (st + 1) * 128],
                        qT[jp][0:64, tch * 512 : (tch + 1) * 512],
                        start=True,
                        stop=True,
                        tile_position=(0, 0),
                    )
                    nc.tensor.matmul(
                        s_b[:],
                        kT[jp][64:128, st * 128 : (st + 1) * 128],
                        qT[jp][64:128, tch * 512 : (tch + 1) * 512],
                        start=True,
                        stop=True,
                        tile_position=(64, 0),
                    )
                    ea = pexp.tile([128, 512], F32R, name="exp_a")
                    eb = pexp.tile([128, 512], F32R, name="exp_b")
                    nc.scalar.activation(
                        out=ea[:], in_=s_a[:], func=AF.Exp, scale=SCALE
                    )
                    nc.scalar.activation(
                        out=eb[:], in_=s_b[:], func=AF.Exp, scale=SCALE
                    )
                    for hh, e_sb, o_key in ((2 * jp, ea, 0), (2 * jp + 1, eb, 1)):
                        lhsT = vext[st][:].rearrange("p h d -> p (h d)")[
                            :, hh * 65 : (hh + 1) * 65
                        ]
                        nc.tensor.matmul(
                            o_ps[o_key][0:65, :],
                            lhsT,
                            e_sb[:],
                            start=(st == 0),
                            stop=(st == NT - 1),
                        )
                for o_key, rowbase in ((0, 0), (1, 64)):
                    rec = pnorm.tile([1, 512], F32, name="recip")
                    nc.vector.reciprocal(out=rec[:], in_=o_ps[o_key][64:65, :])
                    bcast = pnorm.tile([64, 512], F32, name="bcast")
                    nc.gpsimd.partition_broadcast(bcast[:], rec[:])
                    nc.vector.tensor_mul(
                        out=oT[jp][
                            rowbase : rowbase + 64, tch * 512 : (tch + 1) * 512
                        ],
                        in0=o_ps[o_key][0:64, :],
                        in1=bcast[:],
                    )
        pnorm.release()
        pexp.release()
        p_vext.release()
        p_qk.release()

        # ---------------- phase 3: proj + residual + LN2 ----------------
        p_h2 = tc.alloc_tile_pool(name="p_h2", bufs=1)
        p_h2T = tc.alloc_tile_pool(name="p_h2T", bufs=1)
        h2_t = []
        h2T = [p_h2T.tile([128, T], F32R, name=f"h2T_{j}") for j in range(KC)]
        for i in range(NT):
            yt = work.tile([128, C], F32, name="yt")
            for n in range(2):
                py = ps.tile([128, 512], F32, name="py", tag="mm", bufs=2)
                for k in range(KC):
                    nc.tensor.matmul(
                        py[:, :384],
                        oT[k][:, i * 128 : (i + 1) * 128],
                        wp_sb[k][:, n * 384 : (n + 1) * 384],
                        start=(k == 0),
                        stop=(k == KC - 1),
                    )
                nc.vector.tensor_add(
                    out=yt[:, n * 384 : (n + 1) * 384],
                    in0=py[:, :384],
                    in1=bpb[:, n * 384 : (n + 1) * 384],
                )
            nc.vector.tensor_add(out=yt[:], in0=yt[:], in1=h_t[i][:])
            # LN2
            stats = work.tile([128, 3, 6], F32, name="stats2")
            for g in range(3):
                nc.vector.bn_stats(
                    out=stats[:, g, :], in_=yt[:, g * 256 : (g + 1) * 256]
                )
            mv = work.tile([128, 2], F32, name="mv2")
            nc.vector.bn_aggr(out=mv[:], in_=stats[:])
            rstd = work.tile([128, 1], F32, name="rstd2")
            nc.scalar.activation(
                out=rstd[:], in_=mv[:, 1:2], func=AF.Sqrt, bias=eps_t[:]
            )
            nc.vector.reciprocal(out=rstd[:], in_=rstd[:])
            h2i = p_h2.tile([128, C], F32, name=f"h2_{i}")
            nc.vector.tensor_scalar(
                out=h2i[:],
                in0=yt[:],
                scalar1=mv[:, 0:1],
                scalar2=rstd[:],
                op0=ALU.subtract,
                op1=ALU.mult,
            )
            nc.vector.tensor_mul(out=h2i[:], in0=h2i[:], in1=g2b[:])
            nc.vector.tensor_add(out=h2i[:], in0=h2i[:], in1=be2b[:])
            h2_t.append(h2i)
            for j in range(KC):
                pst = ps.tile([128, 128], F32, name="pst2", tag="tr", bufs=2)
                nc.tensor.transpose(pst[:], h2i[:, j * 128 : (j + 1) * 128], ident[:])
                nc.scalar.activation(
                    out=h2T[j][:, i * 128 : (i + 1) * 128], in_=pst[:], func=AF.Copy
                )
        p_wp.release()
        p_oT.release()
        p_h.release()

        # ---------------- phase 4: FFN (f-chunked) ----------------
        p_y2 = tc.alloc_tile_pool(name="p_y2", bufs=1)
        p_w1 = tc.alloc_tile_pool(name="p_w1", bufs=2)
        p_w2 = tc.alloc_tile_pool(name="p_w2", bufs=1)
        p_u = tc.alloc_tile_pool(name="p_u", bufs=1)
        y2 = [p_y2.tile([128, C], F32, name=f"y2_{i}") for i in range(NT)]
        for fc in range(NFC):
            w1c = p_w1.tile([128, KC, FCW], F32R, name="w1c", tag="w1c")
            nc.sync.dma_start(
                out=w1c[:],
                in_=w1_d[:, fc * FCW : (fc + 1) * FCW].rearrange(
                    "(ci p) f -> p ci f", p=128
                ),
            )
            u_sb = [
                p_u.tile([128, T], F32R, name=f"u_{fs}", tag=f"u_{fs}")
                for fs in range(6)
            ]
            for fs in range(6):
                for tch in range(2):
                    pu = ps.tile([128, 512], F32, name="pu", tag="mm", bufs=2)
                    for ci in range(KC):
                        nc.tensor.matmul(
                            pu[:],
                            w1c[:, ci, fs * 128 : (fs + 1) * 128],
                            h2T[ci][:, tch * 512 : (tch + 1) * 512],
                            start=(ci == 0),
                            stop=(ci == KC - 1),
                        )
                    nc.scalar.activation(
                        out=u_sb[fs][:, tch * 512 : (tch + 1) * 512],
                        in_=pu[:],
                        func=AF.Relu,
                        bias=b1_sb[:, fc * 6 + fs : fc * 6 + fs + 1],
                    )
            w2c = p_w2.tile([128, 6, C], F32R, name="w2c", tag="w2c")
            nc.sync.dma_start(
                out=w2c[:],
                in_=w2_d[fc * FCW : (fc + 1) * FCW, :].rearrange(
                    "(fs p) c -> p fs c", p=128
                ),
            )
            for i in range(NT):
                for n in range(2):
                    py2 = ps.tile([128, 512], F32, name="py2", tag="mm", bufs=2)
                    for fs in range(6):
                        nc.tensor.matmul(
                            py2[:, :384],
                            u_sb[fs][:, i * 128 : (i + 1) * 128],
                            w2c[:, fs, n * 384 : (n + 1) * 384],
                            start=(fs == 0),
                            stop=(fs == 5),
                        )
                    if fc == 0:
                        nc.scalar.activation(
                            out=y2[i][:, n * 384 : (n + 1) * 384],
                            in_=py2[:, :384],
                            func=AF.Copy,
                        )
                    else:
                        nc.vector.tensor_add(
                            out=y2[i][:, n * 384 : (n + 1) * 384],
                            in0=py2[:, :384],
                            in1=y2[i][:, n * 384 : (n + 1) * 384],
                        )


        # ---------------- final: out = y2 + b2 + h2 ----------------
        for i in range(NT):
            ot = work.tile([128, C], F32, name="ot")
            nc.vector.tensor_add(out=ot[:], in0=y2[i][:], in1=b2b[:])
            nc.vector.tensor_add(out=ot[:], in0=ot[:], in1=h2_t[i][:])
            nc.sync.dma_start(out=out_d[i * 128 : (i + 1) * 128, :], in_=ot[:])

        p_u.release()
        p_w2.release()
        p_w1.release()
        p_y2.release()
        p_h2T.release()
        p_h2.release()
        ps.release()
        work.release()
        consts.release()

    if split_waits:
        nc.finalize()
        split_excess_waits(nc)
    return nc


def kernel(**inputs):
    x = np.asarray(inputs["x"], dtype=np.float32)
    assert x.shape == (B, T, C), x.shape
    shared = {}
    for name in (
        "Wq", "bq", "Wk", "bk", "Wv", "bv", "Wp", "bp",
        "W1", "b1", "W2", "b2", "g1", "beta1", "g2", "beta2",
    ):
        shared[name] = np.ascontiguousarray(np.asarray(inputs[name], dtype=np.float32))

    nc = build_kernel()
    in_maps = [
        {"x": np.ascontiguousarray(x[b]), **shared} for b in range(B)
    ]
    res = run_bass_kernel_spmd(nc, in_maps, list(range(B)))
    out = np.stack([res.results[b]["out"] for b in range(B)], axis=0)
    return out


if __name__ == "__main__":
    rng = np.random.default_rng(0)
    ins = {
        "x": rng.standard_normal((B, T, C), dtype=np.float32),
        "Wq": (rng.standard_normal((H, C, HS)) / np.sqrt(C)).astype(np.float32),
        "bq": np.zeros((H, HS), np.float32),
        "Wk": (rng.standard_normal((H, C, HS)) / np.sqrt(C)).astype(np.float32),
        "bk": np.zeros((H, HS), np.float32),
        "Wv": (rng.standard_normal((H, C, HS)) / np.sqrt(C)).astype(np.float32),
        "bv": np.zeros((H, HS), np.float32),
        "Wp": (rng.standard_normal((C, C)) / np.sqrt(C)).astype(np.float32),
        "bp": np.zeros((C,), np.float32),
        "W1": (rng.standard_normal((C, F)) / np.sqrt(C)).astype(np.float32),
        "b1": np.zeros((F,), np.float32),
        "W2": (rng.standard_normal((F, C)) / np.sqrt(F)).astype(np.float32),
        "b2": np.zeros((C,), np.float32),
        "g1": np.ones((C,), np.float32),
        "beta1": np.zeros((C,), np.float32),
        "g2": np.ones((C,), np.float32),
        "beta2": np.zeros((C,), np.float32),
    }
    out = kernel(**ins)
    print("out", out.shape, out.dtype, float(np.abs(out).mean()))
